# revision 1
# baseline (speedup 1.0000x reference)
"""Trainium2 Bass kernel for nn_Conv1dFFTPrimative.

Computes, per batch signal x[b] (real, length T=65536):
    X = fft(x); Y = (1/8) * fold_8(X * filt); y = ifft_8192(Y)
output stacked [re, im] on the trailing axis.

Method: matmul-based four-step FFT with N1=N2=256, all on TensorE.
Layouts are arranged so that every stage's output is directly consumable
as the next stage's matmul operand (zero transposes):

  A[n1,n2]   = x.reshape(256,256)
  Bt[n2,k1]  = A.T @ F           (S1; lhsT=A per-signal, rhs=F const)
  Bpt        = Bt * W            (twiddle, W[n2,k1]=e^{-2pi i n2 k1/T}, symmetric)
  Ct[k2,k1]  = F.T @ Bpt         (S2; lhsT=F const, rhs=Bpt)
  Ef         = Ct * filtT        (filtT[k2,k1] = filt.reshape(256,256))
  E[k1,n2']  = Ef.T @ M32        (fold + ifft stage 1 fused; M32[k2,n2'] =
                                  e^{+2pi i k2 n2'/32} / (8*8192), const 256x32)
  Ep         = E * itw           (itw[k1,n2'] = e^{+2pi i k1 n2'/8192})
  Y[n1',n2'] = Gb.T @ Ep         (Gb = conj(F) = [Fr, -Fi]; y[n1'*32+n2'] = Y)

Sharding: pure data parallel, 32 signals per core x 8 cores.
"""

import numpy as np

T = 65536
N1 = 256
HALF = 128
NSIG_PER_CORE = 32
N_CORES = 8


def _tables(k):
    """Host-side constant tables for subsample factor k (k=2**sampling_factor)."""
    M = T // k          # output length (8192 for k=8)
    NI = M // N1        # inverse free dim (32 for k=8)
    n = np.arange(N1, dtype=np.float64)
    ph1 = -2.0 * np.pi * np.outer(n, n) / N1
    F = np.cos(ph1) + 1j * np.sin(ph1)                       # DFT 256 (symmetric)
    phw = -2.0 * np.pi * np.outer(n, n) / T
    W = np.cos(phw) + 1j * np.sin(phw)                        # twiddle (symmetric)
    k2 = np.arange(N1, dtype=np.float64)
    ni = np.arange(NI, dtype=np.float64)
    phm = 2.0 * np.pi * np.outer(k2, ni) / NI
    M32 = (np.cos(phm) + 1j * np.sin(phm)) / (k * M)          # fold + I1, w/ scales
    phi = 2.0 * np.pi * np.outer(k2, ni) / M
    ITW = np.cos(phi) + 1j * np.sin(phi)                      # inverse twiddle
    f32 = lambda a: np.ascontiguousarray(a, dtype=np.float32)
    return {
        "fr": f32(F.real), "fi": f32(F.imag), "fin": f32(-F.imag),
        "wr": f32(W.real), "wi": f32(W.imag),
        "m32r": f32(M32.real), "m32i": f32(M32.imag), "m32in": f32(-M32.imag),
        "itwr": f32(ITW.real), "itwi": f32(ITW.imag),
    }, NI


def _split_multi_waits(nc):
    """Walrus in this toolchain accepts only one sync wait per engine
    instruction; hoist extra waits onto preceding same-engine NoOps."""
    import concourse.mybir as mybir

    n_split = 0
    for fn in nc.m.functions:
        for b in fn.blocks:
            out = []
            for i in b.instructions:
                si = i.sync_info
                if (si is not None and si.on_wait and len(si.on_wait) > 1
                        and type(i).__name__ != "InstEventSemaphore"):
                    waits = list(si.on_wait)
                    for w in waits[:-1]:
                        out.append(mybir.InstNoOp(
                            name=f"{i.name}_wsplit{n_split}",
                            engine=i.engine, ins=[], outs=[],
                            sync_info=mybir.SyncInfo(on_wait=[w], on_update=[]),
                            bass_nofuse=True,
                        ))
                        n_split += 1
                    i.sync_info = mybir.SyncInfo(
                        on_wait=[waits[-1]], on_update=list(si.on_update or [])
                    )
                out.append(i)
            b.instructions = out
    return n_split


def _build(n_sig, ni, use_f32r):
    import concourse.bass as bass
    import concourse.mybir as mybir
    from concourse.tile import TileContext

    f32 = mybir.dt.float32
    mmdt = mybir.dt.float32r if use_f32r else mybir.dt.float32

    nc = bass.Bass()
    x_d = nc.declare_dram_parameter("x", [n_sig, T], f32, isOutput=False)
    ft_d = nc.declare_dram_parameter("ft", [N1, N1], f32, isOutput=False)
    cn = {}
    for name in ("fr", "fi", "fin", "wr", "wi"):
        cn[name] = nc.declare_dram_parameter(name, [N1, N1], f32, isOutput=False)
    for name in ("m32r", "m32i", "m32in", "itwr", "itwi"):
        cn[name] = nc.declare_dram_parameter(name, [N1, ni], f32, isOutput=False)
    y_d = nc.declare_dram_parameter("y", [n_sig, 2, N1 * ni], f32, isOutput=True)

    def mm(ap):
        return ap

    with TileContext(nc) as tc:
        with (
            tc.tile_pool(name="const", bufs=1) as cpool,
            tc.tile_pool(name="work", bufs=3) as wpool,
            tc.tile_pool(name="small", bufs=3) as spool,
            tc.tile_pool(name="pbig", bufs=1, space="PSUM") as pbig,
            tc.tile_pool(name="psmall", bufs=2, space="PSUM") as psmall,
        ):
            # ---- constants into SBUF as [128, 2, width] (row r = h*128+p) ----
            cs = {}
            for name, width in (
                ("fr", N1), ("fi", N1), ("fin", N1), ("wr", N1), ("wi", N1),
                ("ftc", N1),
                ("m32r", ni), ("m32i", ni), ("m32in", ni),
                ("itwr", ni), ("itwi", ni),
            ):
                t = cpool.tile([HALF, 2, width], f32, tag=name)
                src = ft_d if name == "ftc" else cn[name]
                nc.sync.dma_start(
                    out=t[:], in_=src.rearrange("(h p) n -> p h n", p=HALF)
                )
                # funnel DMA deps through DVE so downstream instructions
                # only ever wait on one engine (walrus allows 1 wait/instr)
                t2 = cpool.tile([HALF, 2, width], mmdt, tag=name + "_c")
                nc.vector.tensor_copy(t2[:], t[:])
                cs[name] = t2

            xv = x_d.rearrange("s (h p n) -> s h p n", h=2, p=HALF, n=N1)
            yv = y_d.rearrange("s c (h p n) -> s c h p n", h=2, p=HALF, n=ni)

            for s in range(n_sig):
                # ---- load A ----
                a0 = wpool.tile([HALF, 2, N1], f32, tag="a0")
                nc.sync.dma_start(out=a0[:], in_=xv[s].rearrange("h p n -> p h n"))
                a = wpool.tile([HALF, 2, N1], mmdt, tag="a")
                nc.vector.tensor_copy(a[:], a0[:])

                # ---- S1: Bt = A.T @ F  (out [n2, k1], complex) ----
                bt_r = pbig.tile([HALF, 2, N1], f32, tag="bt_r")
                bt_i = pbig.tile([HALF, 2, N1], f32, tag="bt_i")
                for mh in range(2):
                    msl = slice(mh * HALF, (mh + 1) * HALF)
                    for part, tab in ((bt_r, "fr"), (bt_i, "fi")):
                        for kh in range(2):
                            nc.tensor.matmul(
                                part[:, mh, :],
                                mm(a[:, kh, msl]),
                                mm(cs[tab][:, kh, :]),
                                start=(kh == 0), stop=(kh == 1),
                            )

                # ---- twiddle: Bpt = Bt * W (complex x complex) ----
                bp_r = wpool.tile([HALF, 2, N1], mmdt, tag="bp_r")
                bp_i = wpool.tile([HALF, 2, N1], mmdt, tag="bp_i")
                tw_t = wpool.tile([HALF, 2, N1], f32, tag="tw_t")
                nc.vector.tensor_mul(bp_r[:], bt_r[:], cs["wr"][:])
                nc.vector.tensor_mul(tw_t[:], bt_i[:], cs["wi"][:])
                nc.vector.tensor_sub(bp_r[:], bp_r[:], tw_t[:])
                nc.vector.tensor_mul(bp_i[:], bt_r[:], cs["wi"][:])
                nc.vector.tensor_mul(tw_t[:], bt_i[:], cs["wr"][:])
                nc.vector.tensor_add(bp_i[:], bp_i[:], tw_t[:])

                # ---- S2: Ct = F.T @ Bpt  (out [k2, k1], complex) ----
                ct_r = pbig.tile([HALF, 2, N1], f32, tag="ct_r")
                ct_i = pbig.tile([HALF, 2, N1], f32, tag="ct_i")
                for mh in range(2):
                    msl = slice(mh * HALF, (mh + 1) * HALF)
                    steps_r = [("fr", bp_r), ("fin", bp_i)]
                    steps_i = [("fi", bp_r), ("fr", bp_i)]
                    for part, steps in ((ct_r, steps_r), (ct_i, steps_i)):
                        idx = 0
                        for tab, rhs in steps:
                            for kh in range(2):
                                nc.tensor.matmul(
                                    part[:, mh, :],
                                    mm(cs[tab][:, kh, msl]),
                                    mm(rhs[:, kh, :]),
                                    start=(idx == 0), stop=(idx == 3),
                                )
                                idx += 1

                # ---- filter: Ef = Ct * filtT (real filter) ----
                ef_r = wpool.tile([HALF, 2, N1], mmdt, tag="ef_r")
                ef_i = wpool.tile([HALF, 2, N1], mmdt, tag="ef_i")
                nc.vector.tensor_mul(ef_r[:], ct_r[:], cs["ftc"][:])
                nc.vector.tensor_mul(ef_i[:], ct_i[:], cs["ftc"][:])

                # ---- fold + I1: E = Ef.T @ M32  (out [k1, ni], complex) ----
                # e layout: [p, idx, ni], idx = 2*c + mh (c: 0=re, 1=im)
                e = psmall.tile([HALF, 4, ni], f32, tag="e")
                for mh in range(2):
                    msl = slice(mh * HALF, (mh + 1) * HALF)
                    steps_r = [(ef_r, "m32r"), (ef_i, "m32in")]
                    steps_i = [(ef_r, "m32i"), (ef_i, "m32r")]
                    for c, steps in ((0, steps_r), (1, steps_i)):
                        idx = 0
                        for lt, tab in steps:
                            for kh in range(2):
                                nc.tensor.matmul(
                                    e[:, 2 * c + mh, :],
                                    mm(lt[:, kh, msl]),
                                    mm(cs[tab][:, kh, :]),
                                    start=(idx == 0), stop=(idx == 3),
                                )
                                idx += 1

                # ---- inverse twiddle: Ep = E * itw ----
                ep_r = spool.tile([HALF, 2, ni], mmdt, tag="ep_r")
                ep_i = spool.tile([HALF, 2, ni], mmdt, tag="ep_i")
                it_t = spool.tile([HALF, 2, ni], f32, tag="it_t")
                nc.vector.tensor_mul(ep_r[:], e[:, 0:2, :], cs["itwr"][:])
                nc.vector.tensor_mul(it_t[:], e[:, 2:4, :], cs["itwi"][:])
                nc.vector.tensor_sub(ep_r[:], ep_r[:], it_t[:])
                nc.vector.tensor_mul(ep_i[:], e[:, 0:2, :], cs["itwi"][:])
                nc.vector.tensor_mul(it_t[:], e[:, 2:4, :], cs["itwr"][:])
                nc.vector.tensor_add(ep_i[:], ep_i[:], it_t[:])

                # ---- I2: Y = Gb.T @ Ep  (Gb = conj(F): re=fr, im=fin) ----
                y = psmall.tile([HALF, 4, ni], f32, tag="y")
                for mh in range(2):
                    msl = slice(mh * HALF, (mh + 1) * HALF)
                    steps_r = [("fr", ep_r), ("fi", ep_i)]   # fr.T@ep_r - (-fi).T@ep_i
                    steps_i = [("fin", ep_r), ("fr", ep_i)]
                    for c, steps in ((0, steps_r), (1, steps_i)):
                        idx = 0
                        for tab, rhs in steps:
                            for kh in range(2):
                                nc.tensor.matmul(
                                    y[:, 2 * c + mh, :],
                                    mm(cs[tab][:, kh, msl]),
                                    mm(rhs[:, kh, :]),
                                    start=(idx == 0), stop=(idx == 3),
                                )
                                idx += 1

                # ---- store: y[p, 2c+mh, ni] -> dram [s, c, (mh p ni)] ----
                y_sb = spool.tile([HALF, 4, ni], f32, tag="y_sb")
                nc.vector.tensor_copy(y_sb[:], y[:])
                for c in range(2):
                    nc.sync.dma_start(
                        out=yv[s, c].rearrange("h p n -> p h n"),
                        in_=y_sb[:, 2 * c: 2 * c + 2, :],
                    )
    _split_multi_waits(nc)
    return nc


_BUILD_CACHE = {}
USE_F32R = True


def _run_on_trn(x, filt, k):
    from concourse import bass_utils

    tabs, ni = _tables(k)
    key = (NSIG_PER_CORE, ni, USE_F32R)
    if key not in _BUILD_CACHE:
        _BUILD_CACHE[key] = _build(*key)
    nc = _BUILD_CACHE[key]

    B = x.shape[0]
    xs = np.ascontiguousarray(x.reshape(B, T), dtype=np.float32)
    per = B // N_CORES
    ftab = np.ascontiguousarray(filt.reshape(N1, N1), dtype=np.float32)
    in_maps = []
    for c in range(N_CORES):
        m = {"x": xs[c * per:(c + 1) * per], "ft": ftab}
        m.update(tabs)
        in_maps.append(m)
    res = bass_utils.run_bass_kernel_spmd(nc, in_maps, list(range(N_CORES)))
    outs = [res.results[c]["y"] for c in range(N_CORES)]  # [per, 2, 8192]
    yall = np.concatenate(outs, axis=0)                   # [B, 2, 8192]
    return np.ascontiguousarray(np.moveaxis(yall, 1, 2))  # [B, 8192, 2]


def _numpy_fallback(x, filt, k):
    xh = np.fft.fft(x.astype(np.complex64), axis=-1)
    yc = xh * filt
    B, C, _ = yc.shape
    yh = yc.reshape(B, C, k, T // k).mean(axis=2)
    yr = np.fft.ifft(yh, axis=-1)
    return np.stack([yr.real, yr.imag], axis=-1).astype(np.float32)


def kernel(x, filt, sampling_factor):
    x = np.asarray(x)
    filt = np.asarray(filt)
    k = 2 ** int(sampling_factor)
    B, C, Tin = x.shape
    if Tin != T or C != 1 or B % N_CORES != 0 or k != 8:
        return _numpy_fallback(x, filt, k)
    y = _run_on_trn(x.reshape(B, Tin), filt, k)           # [B, 8192, 2]
    return y.reshape(B, 1, T // k, 2).astype(np.float32)



# revision 10
# speedup vs baseline: 1.9612x; 1.9612x over previous
"""Trainium2 Bass kernel for nn_Conv1dFFTPrimative.

Computes, per batch signal x[b] (real, length T=65536):
    X = fft(x); Y = (1/8) * fold_8(X * filt); y = ifft_8192(Y)
output stacked [re, im] on the trailing axis.

Method: matmul-based four-step FFT with N1=N2=256, all on TensorE.
Layouts are arranged so that every stage's output is directly consumable
as the next stage's matmul operand (zero transposes):

  A[n1,n2]   = x.reshape(256,256)
  Bt[n2,k1]  = A.T @ F           (S1; lhsT=A per-signal, rhs=F const)
  Bpt        = Bt * W            (twiddle, W[n2,k1]=e^{-2pi i n2 k1/T}, symmetric)
  Ct[k2,k1]  = F.T @ Bpt         (S2; lhsT=F const, rhs=Bpt)
  Ef         = Ct * filtT        (filtT[k2,k1] = filt.reshape(256,256))
  E[k1,n2']  = Ef.T @ M32        (fold + ifft stage 1 fused; M32[k2,n2'] =
                                  e^{+2pi i k2 n2'/32} / (8*8192), const 256x32)
  Ep         = E * itw           (itw[k1,n2'] = e^{+2pi i k1 n2'/8192})
  Y[n1',n2'] = Gb.T @ Ep         (Gb = conj(F) = [Fr, -Fi]; y[n1'*32+n2'] = Y)

Sharding: pure data parallel, 32 signals per core x 8 cores.
"""

import numpy as np

T = 65536
N1 = 256
HALF = 128
NSIG_PER_CORE = 32
N_CORES = 8


def _tables(k):
    """Host-side constant tables for subsample factor k (k=2**sampling_factor)."""
    M = T // k          # output length (8192 for k=8)
    NI = M // N1        # inverse free dim (32 for k=8)
    n = np.arange(N1, dtype=np.float64)
    ph1 = -2.0 * np.pi * np.outer(n, n) / N1
    F = np.cos(ph1) + 1j * np.sin(ph1)                       # DFT 256 (symmetric)
    phw = -2.0 * np.pi * np.outer(n, n) / T
    W = np.cos(phw) + 1j * np.sin(phw)                        # twiddle (symmetric)
    k2 = np.arange(N1, dtype=np.float64)
    ni = np.arange(NI, dtype=np.float64)
    phm = 2.0 * np.pi * np.outer(k2, ni) / NI
    M32 = (np.cos(phm) + 1j * np.sin(phm)) / (k * M)          # fold + I1, w/ scales
    phi = 2.0 * np.pi * np.outer(k2, ni) / M
    ITW = np.cos(phi) + 1j * np.sin(phi)                      # inverse twiddle
    f32 = lambda a: np.ascontiguousarray(a, dtype=np.float32)
    return {
        "fr": f32(F.real), "fi": f32(F.imag), "fin": f32(-F.imag),
        "wr": f32(W.real), "wi": f32(W.imag),
        "m32r": f32(M32.real), "m32i": f32(M32.imag), "m32in": f32(-M32.imag),
        "m32p1": f32(np.concatenate([M32.real, M32.imag], axis=1)),
        "m32p2": f32(np.concatenate([-M32.imag, M32.real], axis=1)),
        "itwr": f32(ITW.real), "itwi": f32(ITW.imag),
    }, NI


def _split_multi_waits(nc):
    """Walrus in this toolchain accepts only one sync wait per engine
    instruction; hoist extra waits onto preceding same-engine NoOps."""
    import concourse.mybir as mybir

    n_split = 0
    for fn in nc.m.functions:
        for b in fn.blocks:
            out = []
            for i in b.instructions:
                si = i.sync_info
                if (si is not None and si.on_wait and len(si.on_wait) > 1
                        and type(i).__name__ != "InstEventSemaphore"):
                    waits = list(si.on_wait)
                    for w in waits[:-1]:
                        out.append(mybir.InstNoOp(
                            name=f"{i.name}_wsplit{n_split}",
                            engine=i.engine, ins=[], outs=[],
                            sync_info=mybir.SyncInfo(on_wait=[w], on_update=[]),
                            bass_nofuse=True,
                        ))
                        n_split += 1
                    i.sync_info = mybir.SyncInfo(
                        on_wait=[waits[-1]], on_update=list(si.on_update or [])
                    )
                out.append(i)
            b.instructions = out
    return n_split


def _build(n_sig, ni, use_f32r):
    import concourse.bass as bass
    import concourse.mybir as mybir
    from concourse.tile import TileContext

    f32 = mybir.dt.float32
    mmdt = mybir.dt.bfloat16 if use_f32r else mybir.dt.float32

    nc = bass.Bass()
    x_d = nc.declare_dram_parameter("x", [n_sig, T], f32, isOutput=False)
    ft_d = nc.declare_dram_parameter("ft", [N1, N1], f32, isOutput=False)
    cn = {}
    for name in ("fr", "fi", "fin", "wr", "wi"):
        cn[name] = nc.declare_dram_parameter(name, [N1, N1], f32, isOutput=False)
    for name in ("m32r", "m32i", "m32in", "itwr", "itwi"):
        cn[name] = nc.declare_dram_parameter(name, [N1, ni], f32, isOutput=False)
    for name in ("m32p1", "m32p2"):
        cn[name] = nc.declare_dram_parameter(name, [N1, 2 * ni], f32, isOutput=False)
    y_d = nc.declare_dram_parameter("y", [n_sig, 2, N1 * ni], f32, isOutput=True)

    def mm(ap):
        return ap

    with TileContext(nc) as tc:
        with (
            tc.tile_pool(name="const", bufs=1) as cpool,
            tc.tile_pool(name="work", bufs=3) as wpool,
            tc.tile_pool(name="small", bufs=3) as spool,
            tc.tile_pool(name="pbig", bufs=1, space="PSUM") as pbig,
            tc.tile_pool(name="psmall", bufs=2, space="PSUM") as psmall,
        ):
            # ---- constants into SBUF as [128, 2, width] (row r = h*128+p) ----
            cs = {}
            for name, width in (
                ("fr", N1), ("fi", N1), ("fin", N1), ("wr", N1), ("wi", N1),
                ("ftc", N1),
                ("m32r", ni), ("m32i", ni), ("m32in", ni),
                ("m32p1", 2 * ni), ("m32p2", 2 * ni),
                ("itwr", ni), ("itwi", ni),
            ):
                t = cpool.tile([HALF, 2, width], f32, tag=name)
                src = ft_d if name == "ftc" else cn[name]
                nc.sync.dma_start(
                    out=t[:], in_=src.rearrange("(h p) n -> p h n", p=HALF)
                )
                # funnel DMA deps through DVE so downstream instructions
                # only ever wait on one engine (walrus allows 1 wait/instr)
                # elementwise tables stay f32; matmul tables go to mmdt
                cdt = f32 if name in ("wr", "wi", "itwr", "itwi", "ftc") else mmdt
                t2 = cpool.tile([HALF, 2, width], cdt, tag=name + "_c")
                nc.vector.tensor_copy(t2[:], t[:])
                cs[name] = t2

            xv = x_d.rearrange("s (h p n) -> s h p n", h=2, p=HALF, n=N1)
            yv = y_d.rearrange("s c (h p n) -> s c h p n", h=2, p=HALF, n=ni)

            for s in range(n_sig):
                # ---- load A ----
                a0 = wpool.tile([HALF, 2, N1], f32, tag="a0")
                nc.sync.dma_start(out=a0[:], in_=xv[s].rearrange("h p n -> p h n"))
                a = wpool.tile([HALF, 2, N1], mmdt, tag="a")
                nc.scalar.copy(a[:], a0[:])

                # ---- S1: Bt = A.T @ F  (out [n2, k1], complex) ----
                bt_r = pbig.tile([HALF, 2, N1], f32, tag="bt_r")
                bt_i = pbig.tile([HALF, 2, N1], f32, tag="bt_i")
                for mh in range(2):
                    msl = slice(mh * HALF, (mh + 1) * HALF)
                    for part, tab in ((bt_r, "fr"), (bt_i, "fi")):
                        for kh in range(2):
                            nc.tensor.matmul(
                                part[:, mh, :],
                                mm(a[:, kh, msl]),
                                mm(cs[tab][:, kh, :]),
                                start=(kh == 0), stop=(kh == 1),
                            )

                # ---- twiddle: Bpt = Bt * W (complex x complex) ----
                # re on DVE, im on GpSimd (separate temps to keep engines
                # independent)
                bp_r = wpool.tile([HALF, 2, N1], mmdt, tag="bp_r")
                bp_i = wpool.tile([HALF, 2, N1], mmdt, tag="bp_i")
                tw_t = wpool.tile([HALF, 2, N1], f32, tag="tw_t")
                tw_t2 = wpool.tile([HALF, 2, N1], f32, tag="tw_t2")
                nc.vector.tensor_mul(bp_r[:], bt_r[:], cs["wr"][:])
                nc.vector.tensor_mul(tw_t[:], bt_i[:], cs["wi"][:])
                nc.vector.tensor_sub(bp_r[:], bp_r[:], tw_t[:])
                nc.vector.tensor_mul(bp_i[:], bt_r[:], cs["wi"][:])
                nc.vector.tensor_mul(tw_t2[:], bt_i[:], cs["wr"][:])
                nc.vector.tensor_add(bp_i[:], bp_i[:], tw_t2[:])

                # ---- S2: Ct = F.T @ Bpt  (out [k2, k1], complex) ----
                ct_r = pbig.tile([HALF, 2, N1], f32, tag="ct_r")
                ct_i = pbig.tile([HALF, 2, N1], f32, tag="ct_i")
                for mh in range(2):
                    msl = slice(mh * HALF, (mh + 1) * HALF)
                    steps_r = [("fr", bp_r), ("fin", bp_i)]
                    steps_i = [("fi", bp_r), ("fr", bp_i)]
                    for part, steps in ((ct_r, steps_r), (ct_i, steps_i)):
                        idx = 0
                        for tab, rhs in steps:
                            for kh in range(2):
                                nc.tensor.matmul(
                                    part[:, mh, :],
                                    mm(cs[tab][:, kh, msl]),
                                    mm(rhs[:, kh, :]),
                                    start=(idx == 0), stop=(idx == 3),
                                )
                                idx += 1

                # ---- filter: Ef = Ct * filtT (real filter) ----
                ef_r = wpool.tile([HALF, 2, N1], mmdt, tag="ef_r")
                ef_i = wpool.tile([HALF, 2, N1], mmdt, tag="ef_i")
                nc.vector.tensor_mul(ef_r[:], ct_r[:], cs["ftc"][:])
                nc.vector.tensor_mul(ef_i[:], ct_i[:], cs["ftc"][:])

                # ---- fold + I1: E = Ef.T @ M32  (out [k1, ni], complex) ----
                # e2 layout: [p, mh, c, ni]; paired rhs [m32r|m32i] halves
                # the matmul count (out c-pair contiguous per mh)
                e2 = psmall.tile([HALF, 2, 2, ni], f32, tag="e2")
                for mh in range(2):
                    msl = slice(mh * HALF, (mh + 1) * HALF)
                    idx = 0
                    for lt, tab in ((ef_r, "m32p1"), (ef_i, "m32p2")):
                        for kh in range(2):
                            nc.tensor.matmul(
                                e2[:, mh, :, :],
                                mm(lt[:, kh, msl]),
                                mm(cs[tab][:, kh, :]),
                                start=(idx == 0), stop=(idx == 3),
                            )
                            idx += 1

                # ---- inverse twiddle: Ep = E * itw ----
                # re on DVE, im on GpSimd
                ep_r = spool.tile([HALF, 2, ni], mmdt, tag="ep_r")
                ep_i = spool.tile([HALF, 2, ni], mmdt, tag="ep_i")
                it_t = spool.tile([HALF, 2, ni], f32, tag="it_t")
                it_t2 = spool.tile([HALF, 2, ni], f32, tag="it_t2")
                nc.vector.tensor_mul(ep_r[:], e2[:, :, 0, :], cs["itwr"][:])
                nc.vector.tensor_mul(it_t[:], e2[:, :, 1, :], cs["itwi"][:])
                nc.vector.tensor_sub(ep_r[:], ep_r[:], it_t[:])
                nc.vector.tensor_mul(ep_i[:], e2[:, :, 0, :], cs["itwi"][:])
                nc.vector.tensor_mul(it_t2[:], e2[:, :, 1, :], cs["itwr"][:])
                nc.vector.tensor_add(ep_i[:], ep_i[:], it_t2[:])

                # ---- I2: Y = Gb.T @ Ep  (Gb = conj(F): re=fr, im=fin) ----
                y = psmall.tile([HALF, 4, ni], f32, tag="y")
                for mh in range(2):
                    msl = slice(mh * HALF, (mh + 1) * HALF)
                    steps_r = [("fr", ep_r), ("fi", ep_i)]   # fr.T@ep_r - (-fi).T@ep_i
                    steps_i = [("fin", ep_r), ("fr", ep_i)]
                    for c, steps in ((0, steps_r), (1, steps_i)):
                        idx = 0
                        for tab, rhs in steps:
                            for kh in range(2):
                                nc.tensor.matmul(
                                    y[:, 2 * c + mh, :],
                                    mm(cs[tab][:, kh, msl]),
                                    mm(rhs[:, kh, :]),
                                    start=(idx == 0), stop=(idx == 3),
                                )
                                idx += 1

                # ---- store: y[p, 2c+mh, ni] -> dram [s, c, (mh p ni)] ----
                y_sb = spool.tile([HALF, 4, ni], f32, tag="y_sb")
                nc.scalar.copy(y_sb[:], y[:])
                for c in range(2):
                    nc.sync.dma_start(
                        out=yv[s, c].rearrange("h p n -> p h n"),
                        in_=y_sb[:, 2 * c: 2 * c + 2, :],
                    )
    _split_multi_waits(nc)
    return nc


_BUILD_CACHE = {}
USE_F32R = True


def _run_on_trn(x, filt, k):
    from concourse import bass_utils

    tabs, ni = _tables(k)
    key = (NSIG_PER_CORE, ni, USE_F32R)
    if key not in _BUILD_CACHE:
        _BUILD_CACHE[key] = _build(*key)
    nc = _BUILD_CACHE[key]

    B = x.shape[0]
    xs = np.ascontiguousarray(x.reshape(B, T), dtype=np.float32)
    per = B // N_CORES
    ftab = np.ascontiguousarray(filt.reshape(N1, N1), dtype=np.float32)
    in_maps = []
    for c in range(N_CORES):
        m = {"x": xs[c * per:(c + 1) * per], "ft": ftab}
        m.update(tabs)
        in_maps.append(m)
    res = bass_utils.run_bass_kernel_spmd(nc, in_maps, list(range(N_CORES)))
    outs = [res.results[c]["y"] for c in range(N_CORES)]  # [per, 2, 8192]
    yall = np.concatenate(outs, axis=0)                   # [B, 2, 8192]
    return np.ascontiguousarray(np.moveaxis(yall, 1, 2))  # [B, 8192, 2]


def _numpy_fallback(x, filt, k):
    xh = np.fft.fft(x.astype(np.complex64), axis=-1)
    yc = xh * filt
    B, C, _ = yc.shape
    yh = yc.reshape(B, C, k, T // k).mean(axis=2)
    yr = np.fft.ifft(yh, axis=-1)
    return np.stack([yr.real, yr.imag], axis=-1).astype(np.float32)


def kernel(x, filt, sampling_factor):
    x = np.asarray(x)
    filt = np.asarray(filt)
    k = 2 ** int(sampling_factor)
    B, C, Tin = x.shape
    if Tin != T or C != 1 or B % N_CORES != 0 or k != 8:
        return _numpy_fallback(x, filt, k)
    y = _run_on_trn(x.reshape(B, Tin), filt, k)           # [B, 8192, 2]
    return y.reshape(B, 1, T // k, 2).astype(np.float32)

